# revision 1
# baseline (speedup 1.0000x reference)
"""Trainium2 Bass kernel: 8-head MultiHeadAttention (B=4, N=2048, E=512).

Sharding: 8 cores = 4 batches x 2 query-halves (data parallel). Each core
computes K/V for its whole batch (keys ordered own-half-first, other-half
second -- softmax is invariant to key permutation as long as K and V agree),
attention for its 1024 queries x all 8 heads, and its slice of the output
projection. No collectives; the host concatenates the 8 [1024, 512] slices.

Device-side design:
  - Inputs/weights/activations are bfloat16 (halves input DMA vs f32, same
    PE stream rate, enables fast weight loads); PSUM accumulation is fp32.
    Measured end-to-end relative error ~4e-3 (limit 2e-2).
  - Projections produce feature-major tensors (K^T/Q^T: [head*64+d, tok]) so
    attention scores are computed directly as S^T = K' @ Q^T with keys on
    PSUM partitions (the 1/sqrt(64) scale is folded into wk/bk on the host).
  - Head pairs are processed together: both heads' scores for a q-half go
    into one [128,1024] PSUM tile at PE row groups (0,0)/(64,0), so the two
    K=64 matmuls run concurrently in the array, and a single exp (free dim
    1024, straight out of PSUM) covers both heads.
  - exp is split across TWO engines to beat the scalar engine's ~110us solo
    floor: most tiles run exact exp on the scalar engine (bf16 out); k-chunks
    in DVE_KS run a Schraudolph fast-exp on the vector engine (one fused
    (s*A)+B tensor_scalar in fp32 -- the low 16 bits of each f32 word ARE
    the bf16 exp approximation, read back via a stride-2 bitcast view).
    Softmax normalization cancels most of the ~3% sawtooth error.
  - V is produced token-major with a fused ones-column (V' = [V_h | 1]) so
    the attention-output matmul also yields the softmax denominators free.
  - Normalization multiplies ctx rows by reciprocal denominators broadcast
    across partitions with a K=1 matmul (ones[1,64]^T @ recip[1,q]).
  - Scheduling: the in-order PE stream is kept fed by emitting the next
    pair's K/Q projections, the previous pair's normalization, and the
    previous pair's partial output projection as "fillers" inside the
    current pair's kc loop; DMA loads are ordered to match first use.
    PSUM budget (8 banks) = 2x [128,1024] score tiles + 4x [128,512] slots
    shared by AV accumulators / projection groups / broadcasts / finals.
"""

import os
import sys

import numpy as np

for _p in ("/opt/trn_rl_repo", "/root/.axon_site/_ro/trn_rl_repo"):
    if os.path.isdir(_p) and _p not in sys.path:
        sys.path.insert(0, _p)

import concourse.bass as bass
from concourse import bacc
import concourse.tile as tile
from concourse import mybir
from concourse.bass_utils import run_bass_kernel_spmd

P = 128          # partitions
E = 512          # embed dim
H = 8            # heads
DH = 64          # head dim
T = 2048         # tokens per batch
NQ = 1024        # queries per core
FC = 4           # contraction chunks (512 / 128)
EC = 4           # output-feature chunks
KC = 16          # key-token chunks (2048 / 128)
B = 4
N_CORES = 8

F32 = mybir.dt.float32
F32R = mybir.dt.float32r
BF16 = mybir.dt.bfloat16
ADD = mybir.AluOpType.add
MUL = mybir.AluOpType.mult
EXP = mybir.ActivationFunctionType.Exp

# Schraudolph fast-exp on the vector engine: z = A*s + B computed in fp32;
# the low 16 bits of z's IEEE bit pattern equal the bf16 encoding of
# ~exp(s) (max rel err ~3%, which softmax normalization mostly cancels --
# measured end-to-end metric contribution ~7e-3 even if ALL tiles use it).
# B = 127*2^7 + 1.5*2^23 + C (C=-6 tunes the sawtooth; integer so B is
# exactly representable at ulp=1).
SCH_A = float(2.0 ** 7 / np.log(2.0))
SCH_B = float(127 * 2 ** 7 + 1.5 * 2 ** 23 - 6.0)
# k-chunk indices (of 16) whose exp runs on the vector engine instead of
# the scalar engine -- balances ACT vs DVE busy time.
DVE_KS = (3, 7, 11, 15)


def build_nc(passes=1, packed=True, dbl_scores=False):
    nc = bacc.Bacc(trn_type="TRN2")

    xq = nc.declare_dram_parameter("xq", [E, NQ], BF16, isOutput=False)
    xo = nc.declare_dram_parameter("xo", [E, NQ], BF16, isOutput=False)
    wqt = nc.declare_dram_parameter("wqt", [E, E], BF16, isOutput=False)
    wkt = nc.declare_dram_parameter("wkt", [E, E], BF16, isOutput=False)
    wvt = nc.declare_dram_parameter("wvt", [E, E], BF16, isOutput=False)
    wot = nc.declare_dram_parameter("wot", [E, E], BF16, isOutput=False)
    bqp = nc.declare_dram_parameter("bqp", [P, EC], F32, isOutput=False)
    bkp = nc.declare_dram_parameter("bkp", [P, EC], F32, isOutput=False)
    bvb = nc.declare_dram_parameter("bvb", [P, E], F32, isOutput=False)
    bob = nc.declare_dram_parameter("bob", [P, E], F32, isOutput=False)
    out = nc.declare_dram_parameter("out", [NQ, E], F32, isOutput=True)

    with tile.TileContext(nc) as tc:
        with (
            tc.tile_pool(name="const", bufs=2) as cp,
            tc.tile_pool(name="attn", bufs=2) as atp,
            tc.tile_pool(name="kq", bufs=2) as kqp,
            # vp is AV-read until the very end of a pass; two buffers let the
            # next pass's V' production start without waiting for that tail
            tc.tile_pool(name="vpool", bufs=2) as vpp,
            # input pools double-buffered: pass k+1's HBM reloads overlap
            # pass k's compute instead of serializing at the pass boundary
            # (also keeps the PE from going HAM-cold between passes)
            tc.tile_pool(name="pin", bufs=2) as pin,
            tc.tile_pool(name="exps", bufs=3) as xsp,
            tc.tile_pool(name="exph", bufs=5) as xhp,
            tc.tile_pool(name="expm", bufs=3) as xmp,
            tc.tile_pool(name="norm", bufs=2) as nrm,
            tc.tile_pool(name="osb", bufs=2) as osb,
            tc.tile_pool(name="psA", bufs=2, space="PSUM") as psA,
            tc.tile_pool(name="psO", bufs=4, space="PSUM") as psO,
        ):
            for _pass in range(passes):
                # ---------- input loads ----------
                # DMA order tracks the PE consumption order (QT0, KT0, vp, ...)
                # so the in-order PE stream never waits long on a load.
                wk_t, xq_t, xo_t, wq_t, wv_t, wo_t = [], [], [], [], [], []
                for f in range(FC):
                    t_ = pin.tile([P, NQ], BF16, name=f"xq{f}", tag=f"xq{f}")
                    nc.sync.dma_start(t_[:, 0:E], xq[f * P:(f + 1) * P, 0:E])
                    xq_t.append(t_)
                for f in range(FC):
                    w = pin.tile([P, E], BF16, name=f"wq{f}", tag=f"wq{f}")
                    nc.sync.dma_start(w, wqt[f * P:(f + 1) * P, :])
                    wq_t.append(w)
                bq_t = cp.tile([P, EC], F32, name="bq", tag="bq")
                nc.sync.dma_start(bq_t, bqp[:, :])
                bk_t = cp.tile([P, EC], F32, name="bk", tag="bk")
                nc.sync.dma_start(bk_t, bkp[:, :])
                for f in range(FC):
                    w = pin.tile([P, E], BF16, name=f"wk{f}", tag=f"wk{f}")
                    nc.sync.dma_start(w, wkt[f * P:(f + 1) * P, :])
                    wk_t.append(w)
                for f in range(FC):
                    nc.sync.dma_start(xq_t[f][:, E:NQ], xq[f * P:(f + 1) * P, E:NQ])
                for f in range(FC):
                    w = pin.tile([P, E], BF16, name=f"wv{f}", tag=f"wv{f}")
                    nc.sync.dma_start(w, wvt[f * P:(f + 1) * P, :])
                    wv_t.append(w)
                bvb_t = cp.tile([P, E], F32, name="bvb", tag="bvb")
                nc.sync.dma_start(bvb_t, bvb[:, :])
                for f in range(FC):
                    t_ = pin.tile([P, NQ], BF16, name=f"xo{f}", tag=f"xo{f}")
                    nc.sync.dma_start(t_[:, 0:E], xo[f * P:(f + 1) * P, 0:E])
                    xo_t.append(t_)
                for f in range(FC):
                    nc.sync.dma_start(xo_t[f][:, E:NQ], xo[f * P:(f + 1) * P, E:NQ])
                for f in range(FC):
                    w = cp.tile([P, E], BF16, name=f"wo{f}", tag=f"wo{f}")
                    nc.sync.dma_start(w, wot[f * P:(f + 1) * P, :])
                    wo_t.append(w)
                bob_t = cp.tile([P, E], F32, name="bob", tag="bob")
                nc.sync.dma_start(bob_t, bob[:, :])
                ones_f = cp.tile([P, DH], F32, name="onesf", tag="onesf")
                nc.vector.memset(ones_f, 1.0)
                ones_t = cp.tile([33, DH], F32R, name="ones", tag="ones")
                nc.vector.tensor_copy(out=ones_t, in_=ones_f[0:33, :])

                # ---------- persistent activation tiles ----------
                vp = [vpp.tile([P, H, DH + 1], BF16, name=f"vp{t}", tag=f"vp{t}")
                      for t in range(KC)]
                ctx = [atp.tile([P, NQ], BF16, name=f"ctx{j}", tag=f"ctx{j}")
                       for j in range(EC)]

                def xcat(f, c0, w):
                    # token columns [c0, c0+w) of concat(xq, xo), feature chunk f
                    if c0 + w <= NQ:
                        return xq_t[f][:, c0:c0 + w]
                    return xo_t[f][:, c0 - NQ:c0 - NQ + w]

                def kt_group(kt_j, j, tcp):
                    ps = psO.tile([P, E], F32, name=f"pk{j}_{tcp}", tag="psO")
                    for f in range(FC):
                        nc.tensor.matmul(
                            ps,
                            (wk_t[f][:, j * P:(j + 1) * P]),
                            (xcat(f, tcp * E, E)),
                            start=(f == 0), stop=(f == FC - 1),
                        )
                    nc.vector.tensor_scalar_add(
                        kt_j[:, tcp * E:(tcp + 1) * E], ps, bk_t[:, j:j + 1])

                def qt_group(qt_j, j, tcp):
                    ps = psO.tile([P, E], F32, name=f"pq{j}_{tcp}", tag="psO")
                    for f in range(FC):
                        nc.tensor.matmul(
                            ps,
                            (wq_t[f][:, j * P:(j + 1) * P]),
                            (xq_t[f][:, tcp * E:(tcp + 1) * E]),
                            start=(f == 0), stop=(f == FC - 1),
                        )
                    nc.vector.tensor_scalar_add(
                        qt_j[:, tcp * E:(tcp + 1) * E], ps, bq_t[:, j:j + 1])

                def emit_vp(t):
                    ps = psO.tile([P, E], F32, name=f"pv{t}", tag="psO")
                    for f in range(FC):
                        nc.tensor.matmul(
                            ps,
                            (xcat(f, t * P, P)),
                            (wv_t[f]),
                            start=(f == 0), stop=(f == FC - 1),
                        )
                    nc.vector.tensor_tensor(
                        vp[t][:, :, 0:DH],
                        ps.rearrange("p (h d) -> p h d", d=DH),
                        bvb_t.rearrange("p (h d) -> p h d", d=DH),
                        ADD,
                    )
                    nc.vector.tensor_copy(
                        out=vp[t][:, :, DH:DH + 1], in_=ones_f[:, 0:H, None])

                def emit_head(h, kt_j, qt_j, sh, outs=None, kcs=range(KC),
                              lazy_vp=False, fillers=()):
                    j, par = h // 2, h % 2
                    fillers = list(fillers)
                    if outs is None:
                        o0 = psO.tile([DH + 1, E], F32, name=f"o0_{h}", tag="psO")
                        o1 = psO.tile([DH + 1, E], F32, name=f"o1_{h}", tag="psO")
                        outs = (o0, o1)
                    r0, r1 = par * DH, (par + 1) * DH
                    for k in kcs:
                        if lazy_vp:
                            emit_vp(k)
                        s = psA.tile([P, NQ], F32, name=f"s{h}_{k}", tag="psA")
                        for qc in range(2):
                            nc.tensor.matmul(
                                s[:, qc * E:(qc + 1) * E],
                                (kt_j[r0:r1, k * P:(k + 1) * P]),
                                (qt_j[r0:r1, qc * E:(qc + 1) * E]),
                                start=True, stop=True,
                                tile_position=(par * DH, 0),
                            )
                        ex = xsp.tile([P, NQ], F32R, name=f"ex{h}_{k}", tag="ex")
                        nc.scalar.activation(ex, s, EXP)
                        for qc in range(2):
                            nc.tensor.matmul(
                                outs[qc],
                                (vp[k][:, h, :]),
                                (ex[:, qc * E:(qc + 1) * E]),
                                start=(k == 0), stop=(k == KC - 1),
                            )
                        if fillers:
                            fillers.pop(0)()
                    while fillers:
                        fillers.pop(0)()
                    if kcs[-1] != KC - 1:
                        return outs
                    for qc, o in enumerate(outs):
                        # softmax denominators (ones-column row) -> row 32*par
                        nc.vector.tensor_copy(
                            out=sh[32 * par:32 * par + 1, qc * E:(qc + 1) * E],
                            in_=o[DH:DH + 1, :])
                        nc.vector.tensor_copy(
                            out=ctx[j][r0:r1, qc * E:(qc + 1) * E], in_=o[0:DH, :])
                    return outs

                def emit_pair(j, kt_j, qt_j, sh, fillers=(), stage0=False):
                    # Heads 2j/2j+1 together: per q-half pass, both heads'
                    # scores go into one [128,1024] PSUM tile at row groups
                    # (0,0)/(64,0) -- the PE runs them concurrently -- and
                    # one exp covers both. AV accumulates per head per pass.
                    fillers = list(fillers)
                    for qc in range(2):
                        oe = psO.tile([DH + 1, E], F32,
                                      name=f"oe{j}_{qc}", tag="psO")
                        oo = psO.tile([DH + 1, E], F32,
                                      name=f"oo{j}_{qc}", tag="psO")
                        exh = {}
                        pre = 5 if (stage0 and qc == 0) else 0
                        for k in range(pre):
                            s = psA.tile([P, NQ], F32,
                                         name=f"sp{j}_{qc}_{k}", tag="psA")
                            for par in range(2):
                                nc.tensor.matmul(
                                    s[:, par * E:(par + 1) * E],
                                    (kt_j[par * DH:(par + 1) * DH,
                                          k * P:(k + 1) * P]),
                                    (qt_j[par * DH:(par + 1) * DH,
                                          qc * E:(qc + 1) * E]),
                                    start=True, stop=True,
                                    tile_position=(par * DH, 0),
                                )
                            ex = xhp.tile([P, NQ], BF16,
                                          name=f"exp{j}_{qc}_{k}", tag="exh")
                            nc.scalar.activation(ex, s, EXP)
                            exh[k] = ex
                        if stage0 and qc == 0:
                            kt_group(kt_j, j, 2)
                            kt_group(kt_j, j, 3)
                        for k in range(pre):
                            if stage0 and qc == 0:
                                emit_vp(k)
                            for par, o in ((0, oe), (1, oo)):
                                nc.tensor.matmul(
                                    o,
                                    (vp[k][:, 2 * j + par, :]),
                                    (exh[k][:, par * E:(par + 1) * E]),
                                    start=(k == 0), stop=False,
                                )
                        for k in range(pre, KC):
                            if stage0 and qc == 0:
                                emit_vp(k)
                            s = psA.tile([P, NQ], F32,
                                         name=f"sp{j}_{qc}_{k}", tag="psA")
                            for _rep in range(2 if dbl_scores else 1):
                                for par in range(2):
                                    nc.tensor.matmul(
                                        s[:, par * E:(par + 1) * E],
                                        (kt_j[par * DH:(par + 1) * DH,
                                              k * P:(k + 1) * P]),
                                        (qt_j[par * DH:(par + 1) * DH,
                                              qc * E:(qc + 1) * E]),
                                        start=True, stop=True,
                                        tile_position=(par * DH, 0),
                                    )
                            if k in DVE_KS and not (stage0 and qc == 0):
                                # vector-engine fast exp: one fused
                                # (s*A)+B; low halves of the f32 words are
                                # the bf16 exp values (read via bitcast).
                                exm = xmp.tile([P, NQ], F32,
                                               name=f"exm{j}_{qc}_{k}",
                                               tag="exm")
                                nc.vector.tensor_scalar(
                                    exm, s, SCH_A, SCH_B, MUL, ADD)
                                exv = exm.bitcast(BF16).rearrange(
                                    "p (n two) -> p n two", two=2)
                                rhs_of = (lambda par, _v=exv:
                                          _v[:, par * E:(par + 1) * E, 0])
                            else:
                                ex = xsp.tile([P, NQ], BF16,
                                              name=f"ex{j}_{qc}_{k}", tag="ex")
                                nc.scalar.activation(ex, s, EXP)
                                rhs_of = (lambda par, _e=ex:
                                          _e[:, par * E:(par + 1) * E])
                            for par, o in ((0, oe), (1, oo)):
                                nc.tensor.matmul(
                                    o,
                                    (vp[k][:, 2 * j + par, :]),
                                    (rhs_of(par)),
                                    start=(k == 0), stop=(k == KC - 1),
                                )
                            if fillers:
                                fillers.pop(0)()
                        while fillers:
                            fillers.pop(0)()
                        for par, o in ((0, oe), (1, oo)):
                            nc.vector.tensor_copy(
                                out=sh[32 * par:32 * par + 1,
                                       qc * E:(qc + 1) * E],
                                in_=o[DH:DH + 1, :])
                            nc.vector.tensor_copy(
                                out=ctx[j][par * DH:(par + 1) * DH,
                                           qc * E:(qc + 1) * E],
                                in_=o[0:DH, :])

                def normalize_fillers(j, sh):
                    # reciprocal of the pair's softmax denominators (rows
                    # 0/32), broadcast across the 64 head-dim partitions via
                    # a K=1 matmul, then scale ctx rows in place. Split into
                    # small fillers so it interleaves with the next pair.
                    rp = nrm.tile([33, NQ], F32R, name=f"rp{j}", tag="rp",
                                  bufs=1)

                    def recip():
                        with nc.allow_low_precision(
                                reason="f32r softmax denominators"):
                            nc.vector.reciprocal(rp, sh)

                    def bcast_mul(par, qc):
                        rb = psO.tile([P, E], F32,
                                      name=f"rb{2 * j + par}_{qc}", tag="psO")
                        nc.tensor.matmul(
                            rb[0:DH, :],
                            (ones_t[32 * par:32 * par + 1, :]),
                            (rp[32 * par:32 * par + 1, qc * E:(qc + 1) * E]),
                            start=True, stop=True,
                        )
                        rows = ctx[j][par * DH:(par + 1) * DH,
                                      qc * E:(qc + 1) * E]
                        nc.vector.tensor_tensor(rows, rows, rb[0:DH, :], MUL)

                    return [recip] + [
                        lambda par=par, qc=qc: bcast_mul(par, qc)
                        for qc in range(2) for par in range(2)]

                def final_fillers(j, store=False):
                    # partial output projection for head-pair j, accumulated
                    # into the 8 SBUF output tiles (bias folded into pair 0).
                    def fpass(qt_i):
                        pf = psO.tile([P, E], F32, name=f"pf{j}_{qt_i}",
                                      tag="psO")
                        nc.tensor.matmul(
                            pf,
                            (ctx[j][:, qt_i * P:(qt_i + 1) * P]),
                            (wo_t[j]),
                            start=True, stop=True,
                        )
                        if j == 0:
                            ot = osb.tile([P, E], F32, name=f"ot{qt_i}",
                                          tag=f"ot{qt_i}")
                            ot_t[qt_i] = ot
                            nc.vector.tensor_tensor(ot, pf, bob_t, ADD)
                        else:
                            ot = ot_t[qt_i]
                            nc.vector.tensor_tensor(ot, ot, pf, ADD)
                        if store:
                            nc.sync.dma_start(
                                out[qt_i * P:(qt_i + 1) * P, :], ot_t[qt_i])

                    return [lambda qt_i=qt_i: fpass(qt_i)
                            for qt_i in range(NQ // P)]

                # ---------- schedule ----------
                # pair 0 is staged against DMA arrival (scores+exp for kc 0-3
                # come before any V'/AV work so the scalar engine starts early);
                # projections for pair j+1, normalize(j-1) and the partial
                # output projection for pair j-1 run as fillers inside pair j's
                # kc loops so neither PE nor ACT stalls at pair boundaries.
                kt_n = [None] * EC
                qt_n = [None] * EC
                sh_n = [None] * EC
                ot_t = [None] * (NQ // P)

                def make_pair_fillers(jn):
                    kt_n[jn] = kqp.tile([P, T], BF16, name=f"kt{jn}", tag="kt")
                    qt_n[jn] = kqp.tile([P, NQ], BF16, name=f"qt{jn}", tag="qt")
                    fs = [lambda tcp=tcp: qt_group(qt_n[jn], jn, tcp)
                          for tcp in range(2)]
                    fs += [lambda tcp=tcp: kt_group(kt_n[jn], jn, tcp)
                           for tcp in range(4)]
                    return fs

                if packed:
                    for j in range(EC):
                        sh_n[j] = nrm.tile([33, NQ], F32, name=f"sh{j}",
                                           tag="sh")
                        nc.vector.memset(sh_n[j], 1.0)
                        fillers = []
                        if j == 0:
                            kt_n[0] = kqp.tile([P, T], BF16, name="kt0",
                                               tag="kt")
                            qt_n[0] = kqp.tile([P, NQ], BF16, name="qt0",
                                               tag="qt")
                            qt_group(qt_n[0], 0, 0)
                            kt_group(kt_n[0], 0, 0)
                            kt_group(kt_n[0], 0, 1)
                            qt_group(qt_n[0], 0, 1)
                        else:
                            fillers += normalize_fillers(j - 1, sh_n[j - 1])
                            fillers += final_fillers(j - 1)
                        if j + 1 < EC:
                            fillers += make_pair_fillers(j + 1)
                        emit_pair(j, kt_n[j], qt_n[j], sh_n[j],
                                  fillers=fillers, stage0=(j == 0))
                    nrm3 = normalize_fillers(EC - 1, sh_n[EC - 1])
                    fin3 = final_fillers(EC - 1, store=True)
                    order = [nrm3[0], nrm3[1], nrm3[2], fin3[0], fin3[1],
                             fin3[2], fin3[3], nrm3[3], nrm3[4], fin3[4],
                             fin3[5], fin3[6], fin3[7]]
                    for f in order:
                        f()
                for j in range(EC if not packed else 0):
                    sh_n[j] = nrm.tile([33, NQ], F32, name=f"sh{j}", tag="sh")
                    nc.vector.memset(sh_n[j], 1.0)
                    if j == 0:
                        kt_n[0] = kqp.tile([P, T], F32R, name="kt0", tag="kt")
                        qt_n[0] = kqp.tile([P, NQ], F32R, name="qt0", tag="qt")
                        # group order tracks DMA arrival: qt tcp0 (xq-h0 + wq),
                        # kt tcp0/1 (wk), qt tcp1 (xq-h1)
                        qt_group(qt_n[0], 0, 0)
                        kt_group(kt_n[0], 0, 0)
                        kt_group(kt_n[0], 0, 1)
                        qt_group(qt_n[0], 0, 1)
                        # head-0 prologue: per-half scores+exp for kc 0-2, no AV
                        # yet -- gets the scalar engine going as early as DMA
                        # allows (exp of q-half 0 only needs xq-h0/wq/wk).
                        o0 = psO.tile([DH + 1, E], F32, name="o0_0", tag="psO")
                        o1 = psO.tile([DH + 1, E], F32, name="o1_0", tag="psO")
                        h0_outs = (o0, o1)
                        exh = {}
                        for qc in range(2):
                            for k in range(3):
                                s = psA.tile([P, E], F32,
                                             name=f"s0_{k}_{qc}", tag="psA")
                                nc.tensor.matmul(
                                    s,
                                    (kt_n[0][0:DH, k * P:(k + 1) * P]),
                                    (qt_n[0][0:DH, qc * E:(qc + 1) * E]),
                                    start=True, stop=True, tile_position=(0, 0),
                                )
                                ex = xhp.tile([P, E], F32R,
                                              name=f"exh{k}_{qc}", tag="exh")
                                nc.scalar.activation(ex, s, EXP)
                                exh[(k, qc)] = ex
                        kt_group(kt_n[0], 0, 2)
                        kt_group(kt_n[0], 0, 3)
                        for k in range(3):
                            emit_vp(k)
                            for qc in range(2):
                                nc.tensor.matmul(
                                    h0_outs[qc],
                                    (vp[k][:, 0, :]),
                                    (exh[(k, qc)]),
                                    start=(k == 0), stop=False,
                                )
                        emit_head(0, kt_n[0], qt_n[0], sh_n[0], outs=h0_outs,
                                  kcs=range(3, KC), lazy_vp=True)
                    else:
                        fs = normalize_fillers(j - 1, sh_n[j - 1])
                        fs += final_fillers(j - 1)
                        emit_head(2 * j, kt_n[j], qt_n[j], sh_n[j], fillers=fs)
                    nxt_fillers = make_pair_fillers(j + 1) if j + 1 < EC else ()
                    emit_head(2 * j + 1, kt_n[j], qt_n[j], sh_n[j],
                              fillers=nxt_fillers)

                if not packed:
                    # tail: last pair's normalize interleaved with the final
                    # projection groups (q-tiles 0-3 only need the qc=0
                    # halves of ctx[3] normalized); stores fold into the
                    # final passes.
                    nrm3 = normalize_fillers(EC - 1, sh_n[EC - 1])
                    fin3 = final_fillers(EC - 1, store=True)
                    order = [nrm3[0], nrm3[1], nrm3[2], fin3[0], fin3[1],
                             fin3[2], fin3[3], nrm3[3], nrm3[4], fin3[4],
                             fin3[5], fin3[6], fin3[7]]
                    for f in order:
                        f()

    nc.compile()
    return nc


_NC = None


def _get_nc():
    global _NC
    if _NC is None:
        _NC = build_nc()
    return _NC


def make_in_maps(q, wq, bq, wk, bk, wv, bv, wo, bo):
    import ml_dtypes
    BF = ml_dtypes.bfloat16
    q = np.asarray(q, np.float32)
    scale = 1.0 / np.sqrt(np.float32(DH))
    shared = dict(
        wqt=np.ascontiguousarray(np.asarray(wq, np.float32).T).astype(BF),
        wkt=np.ascontiguousarray(
            np.asarray(wk, np.float32).T * scale).astype(BF),
        wvt=np.ascontiguousarray(np.asarray(wv, np.float32).T).astype(BF),
        wot=np.ascontiguousarray(np.asarray(wo, np.float32).T).astype(BF),
        bqp=np.ascontiguousarray(np.asarray(bq, np.float32).reshape(EC, P).T),
        bkp=np.ascontiguousarray(
            (np.asarray(bk, np.float32) * scale).reshape(EC, P).T),
        bvb=np.ascontiguousarray(
            np.broadcast_to(np.asarray(bv, np.float32), (P, E))),
        bob=np.ascontiguousarray(
            np.broadcast_to(np.asarray(bo, np.float32), (P, E))),
    )
    in_maps = []
    for c in range(N_CORES):
        b, half = c // 2, c % 2
        xT = q[b].T
        in_maps.append(dict(
            xq=np.ascontiguousarray(
                xT[:, half * NQ:(half + 1) * NQ]).astype(BF),
            xo=np.ascontiguousarray(
                xT[:, (1 - half) * NQ:(2 - half) * NQ]).astype(BF),
            **shared,
        ))
    return in_maps


def assemble(results):
    full = np.empty((B, T, E), np.float32)
    for c in range(N_CORES):
        b, half = c // 2, c % 2
        full[b, half * NQ:(half + 1) * NQ, :] = results[c]["out"]
    return full


def kernel(q, wq, bq, wk, bk, wv, bv, wo, bo):
    in_maps = make_in_maps(q, wq, bq, wk, bk, wv, bv, wo, bo)
    nc = _get_nc()
    res = run_bass_kernel_spmd(nc, in_maps, list(range(N_CORES)))
    return assemble(res.results)

